# revision 1
# baseline (speedup 1.0000x reference)
"""BackwardDecoder Trainium2 kernel.

Sharding: data-parallel over batch (B=32 -> 4/core) for the recurrent scan;
vocab-parallel (V -> 4000/core) for the output projection, with one
AllGather of transposed logits in between.

Host-side algebraic folds:
  - Wf folded: gx2 = Wcomb@ctx + bcomb  (Wcomb = Wx2@Wf), and the ctxs
    output-term uses Wfo = Wo_c@Wf; softmax bias bw dropped; bq folded into
    the K-cache; gate biases folded into GX1/bcomb; emb term of the output
    (emb@Wo_e^T + bo + Wo_c@bf) precomputed as L_emb.
"""

import numpy as np

B, T, S, V = 32, 64, 64, 32000
E, H, U, NH = 512, 512, 1024, 8
D, DV = 64, 128
NC = 8
BL = 4          # local batch
VL = V // NC    # 4000
VCH = 500       # vocab chunk per matmul
F32 = np.float32


def host_precompute(inputs):
    import ml_dtypes
    bf16 = ml_dtypes.bfloat16

    tokens = np.asarray(inputs["tokens"]).astype(np.int64)
    enc_mask = np.asarray(inputs["enc_mask"]).astype(bool)
    enc_out = np.asarray(inputs["enc_out"]).astype(F32)
    embed_w = np.asarray(inputs["embed_w"]).astype(F32)
    g1Wx, g1Wh = np.asarray(inputs["gru1_Wx"], F32), np.asarray(inputs["gru1_Wh"], F32)
    g1bx, g1bh = np.asarray(inputs["gru1_bx"], F32), np.asarray(inputs["gru1_bh"], F32)
    g2Wx, g2Wh = np.asarray(inputs["gru2_Wx"], F32), np.asarray(inputs["gru2_Wh"], F32)
    g2bx, g2bh = np.asarray(inputs["gru2_bx"], F32), np.asarray(inputs["gru2_bh"], F32)
    bridge_W, bridge_b = np.asarray(inputs["bridge_W"], F32), np.asarray(inputs["bridge_b"], F32)
    Wk, bk = np.asarray(inputs["Wk"], F32), np.asarray(inputs["bk"], F32)
    Wq, bq = np.asarray(inputs["Wq"], F32), np.asarray(inputs["bq"], F32)
    Ww = np.asarray(inputs["Ww"], F32)
    Wf, bfv = np.asarray(inputs["Wf"], F32), np.asarray(inputs["bf"], F32)
    Wo, bo = np.asarray(inputs["Wo"], F32), np.asarray(inputs["bo"], F32)

    enc = np.transpose(enc_out, (1, 0, 2))                    # [B,S,U]
    lengths = S - enc_mask.sum(axis=1)
    fwd_n = enc.reshape(B, S, 2, U // 2)[np.arange(B), lengths - 1, 0]
    h0 = np.tanh(fwd_n @ bridge_W.T + bridge_b)               # [B,H]

    emb = embed_w[tokens]                                     # [B,T,E]
    WoE, WoH, WoC = Wo[:, :E], Wo[:, E:E + H], Wo[:, E + H:]
    L_emb = emb @ WoE.T + (bo + WoC @ bfv)                    # [B,T,512]
    bias1 = np.concatenate([g1bx[:2 * H] + g1bh[:2 * H], g1bx[2 * H:]])
    GX1 = emb @ g1Wx.T + bias1                                # [B,T,1536]

    Wcomb = g2Wx @ Wf
    bcomb = g2Wx @ bfv + g2bx
    bcomb[:2 * H] += g2bh[:2 * H]
    Wfo = WoC @ Wf                                            # [512,1024]

    K4 = (enc.reshape(B * S, U) @ Wk.T + bk).reshape(B, S, NH, D)
    K4 = np.transpose(K4, (0, 2, 1, 3)) + bq.reshape(NH, 1, D)  # [B,NH,S,D]
    val = enc.reshape(B, S, NH, DV)                           # [B,S,NH,DV]

    def pack_stream(W):
        """gate weight [3C, K] -> rhs stream [128, K/128 * C/128 * 384]."""
        C3, K = W.shape
        C = C3 // 3
        WT = W.T
        out = np.empty((128, K // 128, C // 128, 384), dtype=F32)
        for kt in range(K // 128):
            rows = WT[kt * 128:(kt + 1) * 128]
            for m in range(C // 128):
                out[:, kt, m, 0:128] = rows[:, m * 128:(m + 1) * 128]
                out[:, kt, m, 128:256] = rows[:, C + m * 128:C + (m + 1) * 128]
                out[:, kt, m, 256:384] = rows[:, 2 * C + m * 128:2 * C + (m + 1) * 128]
        return out.reshape(128, -1)

    W1p = pack_stream(g1Wh)                                   # [128,6144]
    W2p = pack_stream(g2Wh)                                   # [128,6144]
    WCp = pack_stream(Wcomb)                                  # [128,12288]
    WQp = Wq.T.reshape(4, 128, 4, 128).transpose(1, 0, 2, 3).reshape(128, -1)

    GB2 = np.zeros((4, 4, 384), dtype=F32)
    for m in range(4):
        GB2[:, m, 0:128] = bcomb[m * 128:(m + 1) * 128]
        GB2[:, m, 128:256] = bcomb[512 + m * 128:512 + (m + 1) * 128]
        GB2[:, m, 256:384] = bcomb[1024 + m * 128:1024 + (m + 1) * 128]
    GB2 = GB2.reshape(4, -1)

    BHN = np.zeros((4, 4, 256), dtype=F32)
    for m in range(4):
        BHN[:, m, 0:128] = g1bh[2 * H + m * 128:2 * H + (m + 1) * 128]
        BHN[:, m, 128:256] = g2bh[2 * H + m * 128:2 * H + (m + 1) * 128]
    BHN = BHN.reshape(4, -1)

    WwPar = np.zeros((128, 2), dtype=F32)
    WwPar[0:64, 0] = Ww[0]
    WwPar[64:128, 1] = Ww[0]

    WOHp = WoH.T.reshape(4, 128, 512).transpose(1, 0, 2).reshape(128, -1)
    WFOp = Wfo.T.reshape(8, 128, 512).transpose(1, 0, 2).reshape(128, -1)

    mask_any = bool(enc_mask.any())
    m01 = np.where(enc_mask, 0.0, 1.0).astype(F32)

    shared = dict(W1p=W1p, WQp=WQp, W2p=W2p, WCp=WCp, WwPar=WwPar,
                  WOHp=WOHp, WFOp=WFOp)
    per_core = []
    for c in range(NC):
        bs = slice(c * BL, (c + 1) * BL)
        gxc = GX1[bs]                                         # [4,T,1536]
        gx1 = np.zeros((T, 4, 4, 384), dtype=F32)             # [t, b, m, 384]
        for bb in range(BL):
            for m in range(4):
                gx1[:, bb, m, 0:128] = gxc[bb, :, m * 128:(m + 1) * 128]
                gx1[:, bb, m, 128:256] = gxc[bb, :, 512 + m * 128:512 + (m + 1) * 128]
                gx1[:, bb, m, 256:384] = gxc[bb, :, 1024 + m * 128:1024 + (m + 1) * 128]
        K4c = K4[bs]                                          # [4,NH,S,D]
        kc = np.zeros((128, 4, BL, S), dtype=F32)
        for cc in range(4):
            for p in range(128):
                hd = cc * 128 + p
                kc[p, cc] = K4c[:, hd // D, :, hd % D]
        vl = np.transpose(val[bs], (1, 0, 2, 3))              # [S,4,NH,DV]
        h0c = h0[bs]
        h0T = np.zeros((128, 16), dtype=F32)
        h0blk = np.zeros((4, 512), dtype=F32)
        for bb in range(BL):
            for kt in range(4):
                h0T[:, 4 * kt + bb] = h0c[bb, kt * 128:(kt + 1) * 128]
                h0blk[bb, kt * 128:(kt + 1) * 128] = h0c[bb, kt * 128:(kt + 1) * 128]
        lec = L_emb[bs]                                       # [4,T,512]
        # LET [128, mo, tok(t,b)]: oc = mo*128+p ; tok col = t*4+b
        let = np.transpose(lec, (2, 1, 0)).reshape(4, 128, T * BL)
        let = let.transpose(1, 0, 2).reshape(128, -1)
        es = embed_w[c * VL:(c + 1) * VL]
        embt = es.T.reshape(4, 128, VL).transpose(1, 0, 2).reshape(128, -1)
        m01p = np.broadcast_to(m01[bs][None, None], (2, 4, BL, S)).reshape(2, -1).copy()
        d = dict(shared)
        d.update(GX1=gx1.reshape(T, -1), Kc=kc.reshape(128, -1),
                 VAL=vl.reshape(S, -1), h0T=h0T, h0blk=h0blk,
                 LET=let, EMBT=embt, M01=m01p, GB2=GB2, BHN=BHN)
        per_core.append({k: np.ascontiguousarray(v) if k in ("GB2", "BHN")
                         else np.ascontiguousarray(v.astype(bf16))
                         for k, v in d.items()})
    return per_core, mask_any


SHAPES = dict(
    W1p=(128, 6144), WQp=(128, 2048), W2p=(128, 6144), WCp=(128, 12288),
    GB2=(4, 1536), WwPar=(128, 2), WOHp=(128, 2048), WFOp=(128, 4096),
    GX1=(T, 16 * 384), Kc=(128, 1024), VAL=(S, 4096),
    h0T=(128, 16), h0blk=(4, 512), LET=(128, 4 * BL * T),
    EMBT=(128, 4 * VL), M01=(2, 1024), BHN=(4, 1024),
)


def build_bass(mask_any):
    import concourse.mybir as mybir
    import concourse.tile as tile
    from concourse import bacc
    from concourse.masks import make_identity

    BF = mybir.dt.bfloat16
    FP = mybir.dt.float32
    AF = mybir.ActivationFunctionType

    nc = bacc.Bacc("TRN2", target_bir_lowering=False)
    din = {}
    for name, shp in SHAPES.items():
        dt = FP if name in ("GB2", "BHN") else BF
        din[name] = nc.dram_tensor(name, shp, dt, kind="ExternalInput")
    out_d = nc.dram_tensor("out_full", (B * T, VL), FP, kind="ExternalOutput")

    from contextlib import ExitStack
    with tile.TileContext(nc) as tc:
        es = ExitStack()
        pool = es.enter_context(tc.tile_pool(name="main", bufs=1))
        psump = es.enter_context(tc.tile_pool(name="ps", bufs=1, space="PSUM"))
        dram = es.enter_context(tc.tile_pool(name="dram", bufs=1, space="DRAM"))

        def load(name, dtype=BF):
            t = pool.tile(list(SHAPES[name]), dtype, tag=name)
            nc.sync.dma_start(t[:, :], din[name][:, :])
            return t

        W1, WQ, W2, WC = load("W1p"), load("WQp"), load("W2p"), load("WCp")
        GB2 = load("GB2", FP)
        BHN = load("BHN", FP)
        WwP, WOH, WFO = load("WwPar"), load("WOHp"), load("WFOp")
        Kc, VAL = load("Kc"), load("VAL")
        LET, EMBT = load("LET"), load("EMBT")
        h0T, h0blk = load("h0T"), load("h0blk")
        M01 = load("M01")

        ident = pool.tile([128, 128], BF, tag="ident")
        make_identity(nc, ident)

        hsT = pool.tile([128, 4 * (T + 1) * 4], BF, tag="hsT")   # (kt,t,b)
        ctxT = pool.tile([128, 8 * T * 4], BF, tag="ctxT")       # (h,t,b)
        hb0 = pool.tile([4, 512], BF, tag="hblk0", name="hb0")
        hb1 = pool.tile([4, 512], BF, tag="hblk1", name="hb1")
        hb = [hb0, hb1]
        nc.vector.tensor_copy(hb[0][:, :], h0blk[:, :])
        nc.vector.tensor_copy(
            hsT[:].rearrange("p (kt t b) -> p kt t b", kt=4, t=T + 1)[:, :, 0, :],
            h0T[:].rearrange("p (kt b) -> p kt b", kt=4))

        def hs_cols(kt, t):
            o = (kt * (T + 1) + t) * 4
            return slice(o, o + 4)

        gxa = pool.tile([4, 1536], BF, tag="gxa", name="gxa")
        gxb = pool.tile([4, 1536], BF, tag="gxb", name="gxb")
        gxt = [gxa, gxb]
        psA = psump.tile([4, 2048], FP, tag="psA", name="psA")

        def reg(ap, lo, hi):  # [4, (m, X)] strided-free slice of a (m,512|384|...) packed AP
            return ap

        for t in range(T):
            # prefetch this step's GX1 slice (scattered rows 32m+b)
            gx = gxt[t % 2]
            nc.sync.dma_start(
                gx[:, :],
                din["GX1"][t:t + 1, :].rearrange("o (b c) -> (o b) c", b=4))

            # ---------- gru1 (+ q region) ----------
            # psum ps1 [128, 512]: rz 0:256 | n 256:384 | q 384:512
            # rows 32m+b; each region's accumulation group is contiguous.
            for m in range(4):
                for kt in range(4):
                    base = (kt * 4 + m) * 384
                    nc.tensor.matmul(psA[:, 512 * m:512 * m + 256],
                                     hsT[:, hs_cols(kt, t)],
                                     W1[:, base:base + 256],
                                     start=(kt == 0), stop=(kt == 3))
                for kt in range(4):
                    base = (kt * 4 + m) * 384
                    nc.tensor.matmul(psA[:, 512 * m + 256:512 * m + 384],
                                     hsT[:, hs_cols(kt, t)],
                                     W1[:, base + 256:base + 384],
                                     start=(kt == 0), stop=(kt == 3))
            psAv = psA[:].rearrange("p (m x) -> p m x", m=4)
            gxv = gx[:].rearrange("p (m x) -> p m x", m=4)
            rza = pool.tile([4, 1024], BF, tag="rza")
            nc.vector.tensor_add(rza[:].rearrange("p (m x) -> p m x", m=4),
                                 psAv[:, :, 0:256], gxv[:, :, 0:256])
            sg1 = pool.tile([4, 1024], BF, tag="sg1")
            nc.scalar.activation(sg1[:, :], rza[:, :], AF.Sigmoid)
            sg1v = sg1[:].rearrange("p (m x) -> p m x", m=4)
            hn1 = pool.tile([4, 512], FP, tag="hn1")
            nc.vector.tensor_add(hn1[:].rearrange("p (m x) -> p m x", m=4),
                                 psAv[:, :, 256:384],
                                 BHN[:].rearrange("p (m x) -> p m x", m=4)[:, :, 0:128])
            t1 = pool.tile([4, 512], BF, tag="t1")
            nc.vector.tensor_mul(t1[:, :], hn1[:, :], sg1v[:, :, 0:128])
            na = pool.tile([4, 512], BF, tag="na")
            nc.vector.tensor_add(na[:].rearrange("p (m x) -> p m x", m=4),
                                 t1[:].rearrange("p (m x) -> p m x", m=4),
                                 gxv[:, :, 256:384])
            n1 = pool.tile([4, 512], BF, tag="n1")
            nc.scalar.activation(n1[:, :], na[:, :], AF.Tanh)
            d1 = pool.tile([4, 512], BF, tag="d1")
            nc.vector.tensor_sub(d1[:, :], hb[t % 2][:, :], n1[:, :])
            e1 = pool.tile([4, 512], BF, tag="e1")
            nc.vector.tensor_mul(e1[:, :], d1[:, :], sg1v[:, :, 128:256])
            tmp = pool.tile([4, 512], BF, tag="tmp")
            nc.vector.tensor_add(tmp[:, :], n1[:, :], e1[:, :])

            # tmp^T
            psT = psump.tile([128, 64], BF, tag="psT")  # tT|qT|aT|hT x16
            for kt in range(4):
                nc.tensor.transpose(psT[:, 4 * kt:4 * kt + 4],
                                    tmp[:, 128 * kt:128 * kt + 128],
                                    ident[0:4, 0:4])
            tmpT = pool.tile([128, 16], BF, tag="tmpT")
            nc.vector.tensor_copy(tmpT[:, :], psT[:, 0:16])

            # ---------- q ----------
            for m in range(4):
                for kt in range(4):
                    nc.tensor.matmul(psA[:, 512 * m + 384:512 * m + 512],
                                     tmpT[:, 4 * kt:4 * kt + 4],
                                     WQ[:, (kt * 4 + m) * 128:(kt * 4 + m + 1) * 128],
                                     start=(kt == 0), stop=(kt == 3))
            qb = pool.tile([4, 512], BF, tag="qb")
            nc.vector.tensor_copy(qb[:].rearrange("p (m x) -> p m x", m=4),
                                  psAv[:, :, 384:512])
            for c in range(4):
                nc.tensor.transpose(psT[:, 16 + 4 * c:16 + 4 * c + 4],
                                    qb[:, 128 * c:128 * c + 128], ident[0:4, 0:4])
            qT = pool.tile([128, 16], BF, tag="qT")
            nc.vector.tensor_copy(qT[:, :], psT[:, 16:32])

            # ---------- attention ----------
            arg = pool.tile([128, 1024], BF, tag="arg")
            qbr = qT[:].rearrange("p (c b) -> p c b", c=4).unsqueeze(3) \
                .to_broadcast((128, 4, 4, S))
            nc.vector.tensor_add(
                arg[:].rearrange("p (c b s) -> p c b s", c=4, b=4),
                Kc[:].rearrange("p (c b s) -> p c b s", c=4, b=4), qbr)
            th = pool.tile([128, 1024], BF, tag="th")
            nc.scalar.activation(th[:, :], arg[:, :], AF.Tanh)
            sc = psump.tile([2, 1024], FP, tag="scpo")
            nc.tensor.matmul(sc[:, 0:512], WwP[:, :], th[:, 0:512],
                             start=True, stop=True)
            nc.tensor.matmul(sc[:, 512:1024], WwP[:, :], th[:, 512:1024],
                             start=True, stop=True)
            ex = pool.tile([2, 1024], BF, tag="ex")
            nc.scalar.activation(ex[:, :], sc[:, :], AF.Exp)
            if mask_any:
                nc.vector.tensor_mul(ex[:, :], ex[:, :], M01[:, :])
            Z = pool.tile([2, 16], FP, tag="Z")
            nc.vector.reduce_sum(Z[:, :],
                                 ex[:].rearrange("p (cb s) -> p cb s", s=S),
                                 axis=mybir.AxisListType.X)
            zr = pool.tile([2, 16], FP, tag="zr")
            nc.vector.reciprocal(zr[:, :], Z[:, :])
            at = pool.tile([2, 1024], BF, tag="at")
            zrb = zr[:].rearrange("p (c b) -> p c b", c=4).unsqueeze(3) \
                .to_broadcast((2, 4, 4, S))
            nc.vector.tensor_mul(
                at[:].rearrange("p (c b s) -> p c b s", c=4, b=4),
                ex[:].rearrange("p (c b s) -> p c b s", c=4, b=4), zrb)

            # attn^T: 8 transposes [2,128] -> [128,2]
            for ch in range(8):
                nc.tensor.transpose(psT[:, 32 + 2 * ch:32 + 2 * ch + 2],
                                    at[:, 128 * ch:128 * (ch + 1)],
                                    ident[0:2, 0:2])
            aT = pool.tile([128, 16], BF, tag="aT")
            nc.vector.tensor_copy(aT[:, :], psT[:, 32:48])
            aLo = pool.tile([64, 16], BF, tag="aLo")
            nc.vector.tensor_copy(aLo[:, :], aT[64:128, :])

            # ctx: 32 val-stationary matvecs -> ctx^T [128,(h,b)]
            psc = psump.tile([128, 32], FP, tag="psc")
            for h in range(NH):
                c, par = h // 2, h % 2
                for bb in range(BL):
                    bp, b2 = bb // 2, bb % 2
                    col = (c * 2 + bp) * 2 + par
                    rhs = (aT[0:64, col:col + 1] if b2 == 0
                           else aLo[:, col:col + 1])
                    nc.tensor.matmul(psc[:, 4 * h + bb:4 * h + bb + 1],
                                     VAL[:, (bb * NH + h) * DV:(bb * NH + h + 1) * DV],
                                     rhs, start=True, stop=True)
            cT = pool.tile([128, 32], BF, tag="cT")
            nc.vector.tensor_copy(cT[:, :], psc[:, :])
            nc.vector.tensor_copy(
                ctxT[:].rearrange("p (h t b) -> p h t b", h=8, t=T)[:, :, t, :],
                cT[:].rearrange("p (h b) -> p h b", h=8))

            # ---------- gh2 + gx2 (contiguous groups per region) ----------
            for m in range(4):
                for kt in range(4):
                    base = (kt * 4 + m) * 384
                    nc.tensor.matmul(psA[:, 512 * m:512 * m + 256],
                                     tmpT[:, 4 * kt:4 * kt + 4],
                                     W2[:, base:base + 256],
                                     start=(kt == 0), stop=False)
                for kt in range(8):
                    base = (kt * 4 + m) * 384
                    nc.tensor.matmul(psA[:, 512 * m:512 * m + 256],
                                     cT[:, 4 * kt:4 * kt + 4],
                                     WC[:, base:base + 256],
                                     start=False, stop=(kt == 7))
                for kt in range(4):
                    base = (kt * 4 + m) * 384
                    nc.tensor.matmul(psA[:, 512 * m + 256:512 * m + 384],
                                     tmpT[:, 4 * kt:4 * kt + 4],
                                     W2[:, base + 256:base + 384],
                                     start=(kt == 0), stop=(kt == 3))
                for kt in range(8):
                    base = (kt * 4 + m) * 384
                    nc.tensor.matmul(psA[:, 512 * m + 384:512 * m + 512],
                                     cT[:, 4 * kt:4 * kt + 4],
                                     WC[:, base + 256:base + 384],
                                     start=(kt == 0), stop=(kt == 7))

            # ---------- gru2 ----------
            GB2v = GB2[:].rearrange("p (m x) -> p m x", m=4)
            rza2 = pool.tile([4, 1024], BF, tag="rza2")
            nc.vector.tensor_add(rza2[:].rearrange("p (m x) -> p m x", m=4),
                                 psAv[:, :, 0:256], GB2v[:, :, 0:256])
            sg2 = pool.tile([4, 1024], BF, tag="sg2")
            nc.scalar.activation(sg2[:, :], rza2[:, :], AF.Sigmoid)
            sg2v = sg2[:].rearrange("p (m x) -> p m x", m=4)
            hn2 = pool.tile([4, 512], FP, tag="hn2")
            nc.vector.tensor_add(hn2[:].rearrange("p (m x) -> p m x", m=4),
                                 psAv[:, :, 256:384],
                                 BHN[:].rearrange("p (m x) -> p m x", m=4)[:, :, 128:256])
            t2 = pool.tile([4, 512], BF, tag="t2")
            nc.vector.tensor_mul(t2[:, :], hn2[:, :], sg2v[:, :, 0:128])
            na2a = pool.tile([4, 512], FP, tag="na2a")
            nc.vector.tensor_add(na2a[:].rearrange("p (m x) -> p m x", m=4),
                                 t2[:].rearrange("p (m x) -> p m x", m=4),
                                 psAv[:, :, 384:512])
            na2 = pool.tile([4, 512], BF, tag="na2")
            nc.vector.tensor_add(na2[:].rearrange("p (m x) -> p m x", m=4),
                                 na2a[:].rearrange("p (m x) -> p m x", m=4),
                                 GB2v[:, :, 256:384])
            n2 = pool.tile([4, 512], BF, tag="n2")
            nc.scalar.activation(n2[:, :], na2[:, :], AF.Tanh)
            d2 = pool.tile([4, 512], BF, tag="d2")
            nc.vector.tensor_sub(d2[:, :], tmp[:, :], n2[:, :])
            e2 = pool.tile([4, 512], BF, tag="e2")
            nc.vector.tensor_mul(e2[:, :], d2[:, :], sg2v[:, :, 128:256])
            h2 = hb[(t + 1) % 2]
            nc.vector.tensor_add(h2[:, :], n2[:, :], e2[:, :])

            for kt in range(4):
                nc.tensor.transpose(psT[:, 48 + 4 * kt:48 + 4 * kt + 4],
                                    h2[:, 128 * kt:128 * kt + 128], ident[0:4, 0:4])
            nc.vector.tensor_copy(
                hsT[:].rearrange("p (kt t b) -> p kt t b", kt=4, t=T + 1)[:, :, t + 1, :],
                psT[:].rearrange("p (x kt b) -> p x kt b", x=4, kt=4)[:, 3, :, :])

        # ================= projection =================
        lgT = pool.tile([128, 4 * 256], BF, tag="lgT")
        for mo in range(4):
            plg_full = psump.tile([128, VCH], FP, tag="scpo")
            plg = plg_full[:, 0:256]
            for kt in range(4):
                rhs = hsT[:, (kt * (T + 1) + 1) * 4:(kt * (T + 1) + 1 + T) * 4]
                nc.tensor.matmul(plg[:, :], WOH[:, (kt * 4 + mo) * 128:(kt * 4 + mo + 1) * 128],
                                 rhs, start=(kt == 0), stop=False)
            for kt in range(8):
                rhs = ctxT[:, kt * T * 4:(kt * T + T) * 4]
                nc.tensor.matmul(plg[:, :], WFO[:, (kt * 4 + mo) * 128:(kt * 4 + mo + 1) * 128],
                                 rhs, start=False, stop=(kt == 7))
            la = pool.tile([128, 256], BF, tag="la")
            nc.vector.tensor_add(la[:, :], plg[:, :], LET[:, mo * 256:(mo + 1) * 256])
            nc.scalar.activation(lgT[:, mo * 256:(mo + 1) * 256], la[:, :], AF.Tanh)

        lgF = pool.tile([128, 4 * NC * 256], BF, tag="lgF")  # (mo, r, tk)
        import os as _os
        if _os.environ.get("SKIP_AG"):
            for r in range(NC):
                nc.vector.tensor_copy(
                    lgF[:].rearrange("p (mo r tk) -> p mo r tk", mo=4, r=NC)[:, :, r, :],
                    lgT[:].rearrange("p (mo tk) -> p mo tk", mo=4))
        else:
            ag_in = dram.tile([128, 1024], BF)
            ag_out = dram.tile([NC * 128, 1024], BF, addr_space="Shared")
            nc.gpsimd.dma_start(ag_in[:, :], lgT[:, :])
            nc.gpsimd.collective_compute(
                "AllGather", mybir.AluOpType.bypass,
                ins=[ag_in.opt()], outs=[ag_out.opt()],
                replica_groups=[list(range(NC))],
            )
            for r in range(NC):
                nc.sync.dma_start(
                    lgF[:].rearrange("p (mo r tk) -> p mo r tk", mo=4, r=NC)[:, :, r, :],
                    ag_out[r * 128:(r + 1) * 128, :].rearrange("p (mo tk) -> p mo tk", mo=4))

        # big vocab matmul: out[tok, v], tok tile = (r, half), col tk = t*4+b
        ov = out_d[:].rearrange("(r b h tp) v -> r h tp b v", r=NC, b=BL, h=2)
        for r in range(NC):
            for half in range(2):
                for vv in range(VL // VCH):
                    po = psump.tile([128, VCH], FP, tag="scpo")
                    for kt in range(4):
                        lhs = lgF[:].rearrange("p (mo r tk) -> p mo r tk", mo=4, r=NC)[:, kt, r, 128 * half:128 * (half + 1)]
                        nc.tensor.matmul(po[:, :], lhs,
                                         EMBT[:, kt * VL + vv * VCH:kt * VL + (vv + 1) * VCH],
                                         start=(kt == 0), stop=(kt == 3))
                    ob = pool.tile([128, VCH], FP, tag="ob")
                    nc.vector.tensor_copy(ob[:, :], po[:, :])
                    nc.sync.dma_start(
                        ov[r, half, :, :, vv * VCH:(vv + 1) * VCH],
                        ob[:, :])
        es.close()
    nc.finalize()
    return nc


_CACHE = {}


def kernel(**inputs):
    from concourse.bass_utils import run_bass_kernel_spmd

    per_core, mask_any = host_precompute(inputs)
    key = ("nc", mask_any)
    if key not in _CACHE:
        _CACHE[key] = build_bass(mask_any)
    nc = _CACHE[key]
    res = run_bass_kernel_spmd(nc, per_core, core_ids=list(range(NC)))
    out = np.empty((B * T, V), dtype=F32)
    for c in range(NC):
        out[:, c * VL:(c + 1) * VL] = res.results[c]["out_full"]
    return out.reshape(B, T, V)


if __name__ == "__main__":
    import reference
    ins = {k: np.asarray(v) for k, v in reference.setup_inputs().items()}
    got = kernel(**ins)
    exp = np.asarray(reference.reference(**reference.setup_inputs()))
    err = np.abs(got - exp).max() / (np.abs(exp).max() + 1e-30)
    print("Relative error:", err)



# revision 20
# speedup vs baseline: 2.7267x; 2.7267x over previous
"""BackwardDecoder Trainium2 kernel (v2).

Sharding: data-parallel over batch (B=32 -> 4/core) for the recurrent scan;
vocab-parallel (V -> 4000/core) for the output projection, with AllGathers
of transposed logits (split in two token halves, overlapped with compute).

v2 design (vs v1): everything lives in a transposed layout with gate/hidden
dims on the 128 partitions and (tile, batch) on the free axis, computed by
weight-stationary matmuls (LDW [128,128] + MM N=4 pairs run at ~31ns).  All
sigmoids are rewritten as tanh (sigmoid(x) = (1+tanh(x/2))/2, with the 0.5
factors folded into weights host-side) so the scalar engine never switches
activation-table sets (tanh and exp share `exp_and_others`).  Attention
scores land on partitions {32c, 32c+1} so exp runs as one [98,256] ACT op.
The output projection is computed incrementally (one token per scan step,
filling the PE stall during the attention softmax), and the two AllGathers
fire mid-scan / before the vocab phase so their latency is hidden.

Host-side algebraic folds (as v1): Wf folded into gru2 x-path (Wcomb) and
the ctxs output-term (Wfo); softmax bias bw dropped; bq folded into the
K-cache; gate biases folded into GX1/BC2; emb term of the output
precomputed as L_emb.
"""

import numpy as np

B, T, S, V = 32, 64, 64, 32000
E, H, U, NH = 512, 512, 1024, 8
D, DV = 64, 128
NC = 8
BL = 4          # local batch
VL = V // NC    # 4000
VCH = 500       # vocab chunk per matmul
TH = T // 2     # token half for projection/AllGather
F32 = np.float32


def host_precompute(inputs):
    import ml_dtypes
    bf16 = ml_dtypes.bfloat16

    tokens = np.asarray(inputs["tokens"]).astype(np.int64)
    enc_mask = np.asarray(inputs["enc_mask"]).astype(bool)
    enc_out = np.asarray(inputs["enc_out"]).astype(F32)
    embed_w = np.asarray(inputs["embed_w"]).astype(F32)
    g1Wx, g1Wh = np.asarray(inputs["gru1_Wx"], F32), np.asarray(inputs["gru1_Wh"], F32)
    g1bx, g1bh = np.asarray(inputs["gru1_bx"], F32), np.asarray(inputs["gru1_bh"], F32)
    g2Wx, g2Wh = np.asarray(inputs["gru2_Wx"], F32), np.asarray(inputs["gru2_Wh"], F32)
    g2bx, g2bh = np.asarray(inputs["gru2_bx"], F32), np.asarray(inputs["gru2_bh"], F32)
    bridge_W, bridge_b = np.asarray(inputs["bridge_W"], F32), np.asarray(inputs["bridge_b"], F32)
    Wk, bk = np.asarray(inputs["Wk"], F32), np.asarray(inputs["bk"], F32)
    Wq, bq = np.asarray(inputs["Wq"], F32), np.asarray(inputs["bq"], F32)
    Ww = np.asarray(inputs["Ww"], F32)
    Wf, bfv = np.asarray(inputs["Wf"], F32), np.asarray(inputs["bf"], F32)
    Wo, bo = np.asarray(inputs["Wo"], F32), np.asarray(inputs["bo"], F32)

    enc = np.transpose(enc_out, (1, 0, 2))                    # [B,S,U]
    lengths = S - enc_mask.sum(axis=1)
    fwd_n = enc.reshape(B, S, 2, U // 2)[np.arange(B), lengths - 1, 0]
    h0 = np.tanh(fwd_n @ bridge_W.T + bridge_b)               # [B,H]

    emb = embed_w[tokens]                                     # [B,T,E]
    WoE, WoH, WoC = Wo[:, :E], Wo[:, E:E + H], Wo[:, E + H:]
    L_emb = emb @ WoE.T + (bo + WoC @ bfv)                    # [B,T,512]

    # gru1 x-gates; r,z prescaled by 0.5 (tanh-halving), bh_rz folded
    gx1f = emb @ g1Wx.T + g1bx                                # [B,T,1536]
    gx1f[:, :, :2 * H] = 0.5 * (gx1f[:, :, :2 * H] + g1bh[:2 * H])

    # gru1 h-weights all halved: rz rows for the tanh-halving trick,
    # n rows because r = (1+tau_r)/2 contributes the 1/2 factor
    W1s = 0.5 * g1Wh
    BHN1h = 0.5 * g1bh[2 * H:]                                # [512]

    # gru2: Wcomb = g2Wx@Wf (x-path via ctx); W2 = g2Wh (h-path)
    Wcomb = g2Wx @ Wf                                         # [1536,1024]
    bcomb = g2Wx @ bfv + g2bx
    W2s = 0.5 * g2Wh
    WCs = Wcomb.copy()
    WCs[:2 * H] *= 0.5
    BC2 = bcomb.copy()
    BC2[:2 * H] = 0.5 * (BC2[:2 * H] + g2bh[:2 * H])
    BHN2h = 0.5 * g2bh[2 * H:]                                # [512]
    Wfo = WoC @ Wf                                            # [512,1024]

    # K-cache with bq folded: [B,NH,S,D]
    K4 = (enc.reshape(B * S, U) @ Wk.T + bk).reshape(B, S, NH, D)
    K4 = np.transpose(K4, (0, 2, 1, 3)) + bq.reshape(NH, 1, D)
    val = enc.reshape(B, S, NH, DV)                           # [B,S,NH,DV]

    def packB(W):
        """[Cout, K] -> lhsT tiles [128, ntile_out*ntile_k*128]:
        tile (mo, kt)[p, c] = W[mo*128+c, kt*128+p]."""
        Cout, K = W.shape
        nm, nk = Cout // 128, K // 128
        t = W.reshape(nm, 128, nk, 128)          # [mo, c, kt, p]
        t = t.transpose(3, 0, 2, 1)              # [p, mo, kt, c]
        return np.ascontiguousarray(t.reshape(128, -1))

    W1B = packB(W1s)                                          # [128,12*4*128]
    WQB = packB(Wq)                                           # [128,4*4*128]
    W2B = packB(W2s)                                          # [128,12*4*128]
    WCB = packB(WCs)                                          # [128,12*8*128]
    WOHB = packB(WoH)                                         # [128,4*4*128]
    WFOB = packB(Wfo)                                         # [128,4*8*128]

    # biases as [128, (tile, b)] with b-broadcast (12 tiles for gru2: 8 rz + 4 n)
    BC2B = np.repeat(BC2.reshape(12, 128).T[:, :, None], BL, axis=2).reshape(128, -1)
    BHN1B = BHN1h.reshape(4, 128).T.copy()                    # [128, 4]
    BHN2B = BHN2h.reshape(4, 128).T.copy()                    # [128, 4]

    WwPar = np.zeros((128, 2), dtype=F32)
    WwPar[0:64, 0] = Ww[0]
    WwPar[64:128, 1] = Ww[0]

    mask_any = bool(enc_mask.any())
    m01 = np.where(enc_mask, 0.0, 1.0).astype(F32)            # [B,S]

    shared = dict(W1B=W1B, WQB=WQB, W2B=W2B, WCB=WCB, WwPar=WwPar,
                  WOHB=WOHB, WFOB=WFOB, BC2B=BC2B, BHN1B=BHN1B, BHN2B=BHN2B)
    per_core = []
    for c in range(NC):
        bs = slice(c * BL, (c + 1) * BL)
        # GX1 [128, (t, m=12, b=4)]
        gxc = gx1f[bs]                                        # [4,T,1536]
        gx1 = gxc.reshape(BL, T, 12, 128).transpose(3, 1, 2, 0)  # [p,t,m,b]
        gx1 = gx1.reshape(128, -1)
        # K-cache [128=(2 heads of 64), (c4, b4, s)]
        K4c = K4[bs]                                          # [4,NH,S,D]
        kc = np.zeros((128, 4, BL, S), dtype=F32)
        for cc in range(4):
            for p in range(128):
                hd = cc * 128 + p
                kc[p, cc] = K4c[:, hd // D, :, hd % D]
        # VAL [64(s), (b, h, dv)]
        vl = np.transpose(val[bs], (1, 0, 2, 3))              # [S,b,h,dv]
        # h0T [128, (kt, b)]
        h0c = h0[bs]
        h0T = h0c.reshape(BL, 4, 128).transpose(2, 1, 0).reshape(128, 16)
        # LET [128, (half, mo, t32, b)]
        lec = L_emb[bs]                                       # [4,T,512]
        let = lec.reshape(BL, 2, TH, 4, 128).transpose(4, 1, 3, 2, 0)
        let = let.reshape(128, -1)                            # [128, 2*4*32*4]
        # EMBT [128, (kt, VL)]
        es = embed_w[c * VL:(c + 1) * VL]
        embt = es.T.reshape(4, 128, VL).transpose(1, 0, 2).reshape(128, -1)
        # mask in [98?, (b, s)] layout -> stored [128, 256], rows 32c+par
        m98 = np.zeros((128, BL * S), dtype=F32)
        mrow = m01[bs].reshape(-1)                            # (b,s)
        for cc in range(4):
            m98[32 * cc] = mrow
            m98[32 * cc + 1] = mrow
        d = dict(shared)
        d.update(GX1=gx1, Kc=kc.reshape(128, -1),
                 VAL=vl.reshape(S, -1), h0T=h0T, LET=let, EMBT=embt, M98=m98)
        per_core.append({k: (np.ascontiguousarray(v) if k in ("BHN1B", "BHN2B")
                             else np.ascontiguousarray(v.astype(bf16)))
                         for k, v in d.items()})
    return per_core, mask_any


SHAPES = dict(
    W1B=(128, 6144), WQB=(128, 2048), W2B=(128, 6144), WCB=(128, 12288),
    WOHB=(128, 2048), WFOB=(128, 4096),
    BC2B=(128, 48), BHN1B=(128, 4), BHN2B=(128, 4), WwPar=(128, 2),
    GX1=(128, T * 12 * BL), Kc=(128, 1024), VAL=(S, BL * NH * DV),
    h0T=(128, 16), LET=(128, 2 * 4 * TH * BL), EMBT=(128, 4 * VL),
    M98=(128, 256),
)


def build_bass(mask_any):
    import concourse.mybir as mybir
    import concourse.tile as tile
    from concourse import bacc
    from concourse.masks import make_identity

    BF = mybir.dt.bfloat16
    FP = mybir.dt.float32
    AF = mybir.ActivationFunctionType
    AX = mybir.AxisListType

    nc = bacc.Bacc("TRN2", target_bir_lowering=False)
    din = {}
    for name, shp in SHAPES.items():
        dt = FP if name in ("BHN1B", "BHN2B") else BF
        din[name] = nc.dram_tensor(name, shp, dt, kind="ExternalInput")
    out_d = nc.dram_tensor("out_full", (B * T, VL), FP, kind="ExternalOutput")
    import os as _os
    _dbg = bool(_os.environ.get("DBG_DUMP"))
    if _dbg:
        dbg_h = nc.dram_tensor("dbg_h", (128, 4 * (T + 1) * BL), FP, kind="ExternalOutput")
        dbg_c = nc.dram_tensor("dbg_c", (128, NH * T * BL), FP, kind="ExternalOutput")
        dbg_l = nc.dram_tensor("dbg_l", (128, 2 * 4 * TH * BL), FP, kind="ExternalOutput")

    from contextlib import ExitStack
    with tile.TileContext(nc) as tc:
        es = ExitStack()
        pool = es.enter_context(tc.tile_pool(name="main", bufs=1))
        psump = es.enter_context(tc.tile_pool(name="ps", bufs=1, space="PSUM"))
        dram = es.enter_context(tc.tile_pool(name="dram", bufs=1, space="DRAM"))

        def load(name, dtype=BF):
            t = pool.tile(list(SHAPES[name]), dtype, tag=name)
            nc.sync.dma_start(t[:, :], din[name][:, :])
            return t

        # order: first-needed first
        h0T = load("h0T")
        W1B = load("W1B")
        GX1 = load("GX1")
        BHN1B, BHN2B = load("BHN1B", FP), load("BHN2B", FP)
        WQB, Kc = load("WQB"), load("Kc")
        WwP, VAL = load("WwPar"), load("VAL")
        W2B, WCB = load("W2B"), load("WCB")
        BC2B = load("BC2B")
        WOHB, WFOB, LET = load("WOHB"), load("WFOB"), load("LET")
        EMBT = load("EMBT")
        M98 = load("M98") if mask_any else None

        ident = pool.tile([128, 128], BF, tag="ident")
        make_identity(nc, ident)

        # state/history
        hsT = pool.tile([128, 4 * (T + 1) * BL], BF, tag="hsT")   # (kt,t,b)
        ctxT = pool.tile([128, NH * T * BL], BF, tag="ctxT")      # (h,t,b)
        nc.vector.tensor_copy(
            hsT[:].rearrange("p (kt t b) -> p kt t b", kt=4, t=T + 1)[:, :, 0, :],
            h0T[:].rearrange("p (kt b) -> p kt b", kt=4))

        def hcols(t):   # [128, (kt,b)] strided view of hsT at step t
            return hsT[:].rearrange("p (kt t b) -> p kt t b", kt=4, t=T + 1)[:, :, t, :]

        # psum tiles (tags -> banks); P1 also holds q in cols 48:64
        P1 = psump.tile([128, 64], FP, tag="P1", name="P1")
        P2 = psump.tile([128, 64], FP, tag="P2", name="P2")
        SC = psump.tile([128, 256], FP, tag="SC", name="SC")
        PT = psump.tile([128, 16], BF, tag="PT", name="PT")
        PC = psump.tile([128, 32], FP, tag="PC", name="PC")
        PJ = psump.tile([128, 16], FP, tag="PJ", name="PJ")

        lgT = pool.tile([128, 2 * 4 * TH * BL], BF, tag="lgT")    # (half,mo,t32,b)
        lgTv = lgT[:].rearrange("p (hf mo t b) -> p hf mo t b", hf=2, mo=4, t=TH)
        LETv = LET[:].rearrange("p (hf mo t b) -> p hf mo t b", hf=2, mo=4, t=TH)

        ag_in = [dram.tile([128, 4 * TH * BL], BF, name=f"ag_in{i}")
                 for i in range(2)]
        ag_out = [dram.tile([NC * 128, 4 * TH * BL], BF, addr_space="Shared",
                            name=f"ag_out{i}") for i in range(2)]
        lgA = [pool.tile([128, NC * 4 * TH * BL], BF, tag=f"lgA{i}",
                         name=f"lgA{i}") for i in range(2)]

        def w1(mo, kt):
            return W1B[:, (mo * 4 + kt) * 128:(mo * 4 + kt + 1) * 128]

        def wq(mo, kt):
            return WQB[:, (mo * 4 + kt) * 128:(mo * 4 + kt + 1) * 128]

        def w2(mo, kt):
            return W2B[:, (mo * 4 + kt) * 128:(mo * 4 + kt + 1) * 128]

        def wc(mo, kt):
            return WCB[:, (mo * 8 + kt) * 128:(mo * 8 + kt + 1) * 128]

        GX1v = GX1[:].rearrange("p (t m b) -> p t m b", t=T, m=12)

        tmpT = pool.tile([128, 16], BF, tag="tmpT")     # (kt,b)
        arg = pool.tile([128, 1024], BF, tag="arg")
        th = pool.tile([128, 1024], BF, tag="th")
        ex = pool.tile([128, 256], BF, tag="ex")        # rows 32c+par
        Zr = pool.tile([128, 4], FP, tag="Zr")
        zi = pool.tile([128, 4], FP, tag="zi")
        at = pool.tile([128, 256], BF, tag="at")
        aTs = pool.tile([128, 16], BF, tag="aTs")       # (b2,s) x (c,bp,par)
        aLo = pool.tile([64, 16], BF, tag="aLo")
        cT = pool.tile([128, 32], BF, tag="cT")         # (h,b) -- scratch per step

        # zero SC once so the [98,256] exp never reads uninitialized psum rows
        nc.vector.memset(SC[:, :], 0.0)

        def gru_nonlin(P, gxn_ap, bhnB, h_prev_ap, h_out_ap, pfx):
            """P: psum [128, 64] with rz in cols 0:32 (tiles 0..7 = r0..3,z4..7),
            hn in 48:64.  gxn_ap / h_prev_ap / h_out_ap are [128, 4, 4] views.
            Writes h_out_ap = n + (h_prev - n) * z."""
            Pv = P[:].rearrange("p (m b) -> p m b", m=16)

            def v3(ap):
                return ap[:].rearrange("p (m b) -> p m b", m=4)

            trz = pool.tile([128, 32], BF, tag=pfx + "trz")
            nc.scalar.activation(trz[:, :], P[:, 0:32], AF.Tanh)
            # w = 1 + tau_r ; z = (1+tau_z)/2
            w1p = pool.tile([128, 16], BF, tag=pfx + "w1p")
            nc.vector.tensor_scalar_add(w1p[:, :], trz[:, 0:16], 1.0)
            zg = pool.tile([128, 16], BF, tag=pfx + "zg")
            nc.vector.tensor_scalar(zg[:, :], trz[:, 16:32], 0.5, 0.5,
                                    mybir.AluOpType.mult, mybir.AluOpType.add)
            # hn_half = psum_hn + bhn_half
            hns = pool.tile([128, 16], FP, tag=pfx + "hns")
            nc.vector.tensor_add(v3(hns), Pv[:, 12:16, :],
                                 bhnB[:].rearrange("p m -> p m").unsqueeze(2).to_broadcast((128, 4, BL)))
            t1 = pool.tile([128, 16], BF, tag=pfx + "t1")
            nc.vector.tensor_mul(t1[:, :], hns[:, :], w1p[:, :])
            na = pool.tile([128, 16], BF, tag=pfx + "na")
            nc.vector.tensor_add(v3(na), gxn_ap, v3(t1))
            n1 = pool.tile([128, 16], BF, tag=pfx + "n1")
            nc.scalar.activation(n1[:, :], na[:, :], AF.Tanh)
            d1 = pool.tile([128, 16], BF, tag=pfx + "d1")
            nc.vector.tensor_sub(v3(d1), h_prev_ap, v3(n1))
            e1 = pool.tile([128, 16], BF, tag=pfx + "e1")
            nc.vector.tensor_mul(e1[:, :], d1[:, :], zg[:, :])
            nc.vector.tensor_add(h_out_ap, v3(n1), v3(e1))

        for t in range(T):
            hp = hcols(t)   # [128, kt, b] bf16 view

            # ---------- gru1: rz tiles 0..7 (+GX1 via ident-MM), hn tiles ----------
            for mo in range(8):
                for kt in range(4):
                    nc.tensor.matmul(P1[:, mo * 4:mo * 4 + 4], w1(mo, kt),
                                     hp[:, kt, :], start=(kt == 0), stop=False)
                nc.tensor.matmul(P1[:, mo * 4:mo * 4 + 4], ident[:, :],
                                 GX1v[:, t, mo, :], start=False, stop=True)
            for mo in range(4):
                for kt in range(4):
                    nc.tensor.matmul(P1[:, 48 + mo * 4:48 + mo * 4 + 4],
                                     w1(8 + mo, kt), hp[:, kt, :],
                                     start=(kt == 0), stop=(kt == 3))

            gru_nonlin(P1, GX1v[:, t, 8:12, :], BHN1B, hp,
                       tmpT[:].rearrange("p (kt b) -> p kt b", kt=4), "g1")

            # ---------- q ----------
            for mo in range(4):
                for kt in range(4):
                    nc.tensor.matmul(P1[:, 32 + mo * 4:32 + mo * 4 + 4],
                                     wq(mo, kt),
                                     tmpT[:, kt * 4:kt * 4 + 4],
                                     start=(kt == 0), stop=(kt == 3))

            # ---------- gru2 h-path hn tiles (contiguous, close now) ----------
            for mo in range(4):
                for kt in range(4):
                    nc.tensor.matmul(P2[:, 48 + mo * 4:48 + mo * 4 + 4],
                                     w2(8 + mo, kt), tmpT[:, kt * 4:kt * 4 + 4],
                                     start=(kt == 0), stop=(kt == 3))

            # ---------- attention ----------
            qbr = P1[:].rearrange("p (x c b) -> p x c b", x=4, c=4)[:, 2, :, :] \
                .unsqueeze(3).to_broadcast((128, 4, 4, S))
            nc.vector.tensor_add(
                arg[:].rearrange("p (c b s) -> p c b s", c=4, b=4),
                Kc[:].rearrange("p (c b s) -> p c b s", c=4, b=4), qbr)
            nc.scalar.activation(th[:, :], arg[:, :], AF.Tanh)
            for cc in range(4):
                nc.tensor.matmul(SC[32 * cc:32 * cc + 2, :], WwP[:, :],
                                 th[:, cc * 256:(cc + 1) * 256],
                                 start=True, stop=True, tile_position=(0, 32 * cc))

            # projection of step t-1 (fills PE stall during softmax)
            if t >= 1:
                tp = t - 1
                hf, t32 = tp // TH, tp % TH
                for mo in range(4):
                    for kt in range(4):
                        nc.tensor.matmul(
                            PJ[:, mo * 4:mo * 4 + 4],
                            WOHB[:, (mo * 4 + kt) * 128:(mo * 4 + kt + 1) * 128],
                            hcols(tp + 1)[:, kt, :], start=(kt == 0), stop=False)
                    for kt in range(8):
                        nc.tensor.matmul(
                            PJ[:, mo * 4:mo * 4 + 4],
                            WFOB[:, (mo * 8 + kt) * 128:(mo * 8 + kt + 1) * 128],
                            ctxT[:, (kt * T + tp) * 4:(kt * T + tp) * 4 + 4],
                            start=False, stop=(kt == 7))
                lg1 = pool.tile([128, 16], BF, tag="lg1")
                nc.vector.tensor_add(lg1[:].rearrange("p (m b) -> p m b", m=4),
                                     PJ[:].rearrange("p (m b) -> p m b", m=4),
                                     LETv[:, hf, :, t32, :])
                nc.scalar.activation(lgTv[:, hf, :, t32, :],
                                     lg1[:].rearrange("p (m b) -> p m b", m=4),
                                     AF.Tanh)
                if tp == TH - 1:
                    nc.gpsimd.dma_start(ag_in[0][:, :],
                                        lgTv[:, 0].rearrange("p m t b -> p (m t b)"))
                    nc.gpsimd.collective_compute(
                        "AllGather", mybir.AluOpType.bypass,
                        ins=[ag_in[0].opt()], outs=[ag_out[0].opt()],
                        replica_groups=[list(range(NC))])

            # softmax over s: exp on [98,256]; Z reduce; normalize
            nc.scalar.activation(ex[0:98, :], SC[0:98, :], AF.Exp)
            if mask_any:
                nc.vector.tensor_mul(ex[0:98, :], ex[0:98, :], M98[0:98, :])
            nc.vector.reduce_sum(Zr[0:98, :],
                                 ex[0:98, :].rearrange("p (b s) -> p b s", b=4),
                                 axis=AX.X)
            nc.vector.reciprocal(zi[0:98, :], Zr[0:98, :])
            nc.vector.tensor_mul(
                at[0:98, :].rearrange("p (b s) -> p b s", b=4),
                ex[0:98, :].rearrange("p (b s) -> p b s", b=4),
                zi[0:98, :].unsqueeze(2).to_broadcast((98, 4, S)))

            # attn^T: 8 transposes [2,128] -> [128,2]; cols (c,bp,par)
            for cc in range(4):
                for bp in range(2):
                    nc.tensor.transpose(PT[:, cc * 4 + bp * 2:cc * 4 + bp * 2 + 2],
                                        at[32 * cc:32 * cc + 2, bp * 128:bp * 128 + 128],
                                        ident[32 * cc:32 * cc + 2, 32 * cc:32 * cc + 2],
                                        tile_position=(32 * cc, 0))
            nc.vector.tensor_copy(aTs[:, :], PT[:, :])
            nc.vector.tensor_copy(aLo[:, :], aTs[64:128, :])

            # ctx matvecs -> PC [128,(h,b)]
            for h in range(NH):
                cc, par = h // 2, h % 2
                for b in range(BL):
                    bp, b2 = b // 2, b % 2
                    col = cc * 4 + bp * 2 + par
                    rhs = (aTs[0:64, col:col + 1] if b2 == 0
                           else aLo[:, col:col + 1])
                    nc.tensor.matmul(
                        PC[:, h * 4 + b:h * 4 + b + 1],
                        VAL[:, (b * NH + h) * DV:(b * NH + h + 1) * DV],
                        rhs, start=True, stop=True)
            nc.vector.tensor_copy(cT[:, :], PC[:, :])
            nc.vector.tensor_copy(
                ctxT[:].rearrange("p (h t b) -> p h t b", h=8, t=T)[:, :, t, :],
                cT[:].rearrange("p (h b) -> p h b", h=8))

            # ---------- gru2 rz: W2 + WC + bias, one contiguous group ----------
            for mo in range(8):
                for kt in range(4):
                    nc.tensor.matmul(P2[:, mo * 4:mo * 4 + 4], w2(mo, kt),
                                     tmpT[:, kt * 4:kt * 4 + 4],
                                     start=(kt == 0), stop=False)
                for kt in range(8):
                    nc.tensor.matmul(P2[:, mo * 4:mo * 4 + 4], wc(mo, kt),
                                     cT[:, kt * 4:kt * 4 + 4],
                                     start=False, stop=False)
                nc.tensor.matmul(P2[:, mo * 4:mo * 4 + 4], ident[:, :],
                                 BC2B[:, mo * 4:mo * 4 + 4], start=False, stop=True)
            for mo in range(4):
                for kt in range(8):
                    nc.tensor.matmul(P2[:, 32 + mo * 4:32 + mo * 4 + 4],
                                     wc(8 + mo, kt), cT[:, kt * 4:kt * 4 + 4],
                                     start=(kt == 0), stop=False)
                nc.tensor.matmul(P2[:, 32 + mo * 4:32 + mo * 4 + 4], ident[:, :],
                                 BC2B[:, 32 + mo * 4:32 + mo * 4 + 4],
                                 start=False, stop=True)

            # gru2 nonlin: n-arg = xn (cols 32:48) + hn_half*(1+tau_r)
            Pv2 = P2[:].rearrange("p (m b) -> p m b", m=16)
            gru_nonlin(P2, Pv2[:, 8:12, :], BHN2B, tmpT[:].rearrange("p (kt b) -> p kt b", kt=4),
                       hcols(t + 1), "g2")

        # ---------- final projection step (t=63) + AG1 ----------
        tp = T - 1
        for mo in range(4):
            for kt in range(4):
                nc.tensor.matmul(PJ[:, mo * 4:mo * 4 + 4],
                                 WOHB[:, (mo * 4 + kt) * 128:(mo * 4 + kt + 1) * 128],
                                 hcols(tp + 1)[:, kt, :], start=(kt == 0), stop=False)
            for kt in range(8):
                nc.tensor.matmul(PJ[:, mo * 4:mo * 4 + 4],
                                 WFOB[:, (mo * 8 + kt) * 128:(mo * 8 + kt + 1) * 128],
                                 ctxT[:, (kt * T + tp) * 4:(kt * T + tp) * 4 + 4],
                                 start=False, stop=(kt == 7))
        lg1 = pool.tile([128, 16], BF, tag="lg1")
        nc.vector.tensor_add(lg1[:].rearrange("p (m b) -> p m b", m=4),
                             PJ[:].rearrange("p (m b) -> p m b", m=4),
                             LETv[:, 1, :, TH - 1, :])
        nc.scalar.activation(lgTv[:, 1, :, TH - 1, :],
                             lg1[:].rearrange("p (m b) -> p m b", m=4),
                             AF.Tanh)
        nc.gpsimd.dma_start(ag_in[1][:, :],
                            lgTv[:, 1].rearrange("p m t b -> p (m t b)"))
        nc.gpsimd.collective_compute(
            "AllGather", mybir.AluOpType.bypass,
            ins=[ag_in[1].opt()], outs=[ag_out[1].opt()],
            replica_groups=[list(range(NC))])

        if _dbg:
            hsF = pool.tile([128, 4 * (T + 1) * BL], FP, tag="hsF")
            nc.vector.tensor_copy(hsF[:, :], hsT[:, :])
            nc.sync.dma_start(dbg_h[:, :], hsF[:, :])
            ctF = pool.tile([128, NH * T * BL], FP, tag="ctF")
            nc.vector.tensor_copy(ctF[:, :], ctxT[:, :])
            nc.sync.dma_start(dbg_c[:, :], ctF[:, :])
            lgF_ = pool.tile([128, 2 * 4 * TH * BL], FP, tag="lgF_")
            nc.vector.tensor_copy(lgF_[:, :], lgT[:, :])
            nc.sync.dma_start(dbg_l[:, :], lgF_[:, :])

        # gather results into SBUF: lgA[half][:, r*512 + (mo,t32-in-half? ...)]
        for hf in range(2):
            for r in range(NC):
                nc.sync.dma_start(lgA[hf][:, r * 512:(r + 1) * 512],
                                  ag_out[hf][r * 128:(r + 1) * 128, :])

        # ---------- vocab matmul ----------
        # out rows tok = (r, hf, t32, b); lhsT = lgA[hf][:, r*512 + mo*128 ...]
        ov = out_d[:].rearrange("(r b hf tp) v -> hf r tp b v", r=NC, b=BL, hf=2)
        PO = [psump.tile([128, VCH], FP, tag=f"PO{i}", name=f"PO{i}") for i in range(2)]
        ob = [pool.tile([128, VCH], FP, tag=f"ob{i}", name=f"ob{i}")
              for i in range(4)]
        it = 0
        for hf in range(2):
            for r in range(NC):
                # lhsT cols must be tok=(t32,b)=128; partition p = oc within tile mo
                for vv in range(VL // VCH):
                    po = PO[it % 2]
                    for kt in range(4):
                        nc.tensor.matmul(
                            po[:, :],
                            lgA[hf][:, r * 512 + kt * 128:r * 512 + (kt + 1) * 128],
                            EMBT[:, kt * VL + vv * VCH:kt * VL + (vv + 1) * VCH],
                            start=(kt == 0), stop=(kt == 3))
                    o = ob[it % 4]
                    if it % 2 == 0:
                        nc.vector.tensor_copy(o[:, :], po[:, :])
                    else:
                        nc.scalar.activation(o[:, :], po[:, :], AF.Copy)
                    nc.sync.dma_start(ov[hf, r, :, :, vv * VCH:(vv + 1) * VCH],
                                      o[:, :])
                    it += 1
        es.close()
    nc.finalize()
    return nc


_CACHE = {}


def kernel(**inputs):
    from concourse.bass_utils import run_bass_kernel_spmd

    per_core, mask_any = host_precompute(inputs)
    key = ("nc", mask_any)
    if key not in _CACHE:
        _CACHE[key] = build_bass(mask_any)
    nc = _CACHE[key]
    res = run_bass_kernel_spmd(nc, per_core, core_ids=list(range(NC)))
    out = np.empty((B * T, V), dtype=F32)
    for c in range(NC):
        out[:, c * VL:(c + 1) * VL] = res.results[c]["out_full"]
    return out.reshape(B, T, V)


if __name__ == "__main__":
    import reference
    ins = {k: np.asarray(v) for k, v in reference.setup_inputs().items()}
    got = kernel(**ins)
    exp = np.asarray(reference.reference(**reference.setup_inputs()))
    err = np.abs(got - exp).max() / (np.abs(exp).max() + 1e-30)
    print("Relative error:", err)
